# revision 20
# baseline (speedup 1.0000x reference)
"""Trainium2 Bass kernel for nn_ATVP_router_UNI (moe_routing).

Sharding: output dim D=1536 sharded over 8 cores (192 each). Activations
(x_enc, x_ib) are broadcast in a partition-major transposed layout; the
router MLP is replicated on every core. The expert mean over e commutes
with the linear projection, so each core streams its W_proj slice once
(bf16), reduces over e (split across GpSimd and DVE), and runs 1/7 of the
naive matmul FLOPs in bf16. All heavy streams are cast to bf16 on the
host -- this halves HBM traffic, which is the roofline for this kernel.
The softmax denominator and the 1/7 group-mean scale cancel inside the
final l2 normalization and are folded away. Per-expert biases are
e-reduced on DVE and folded into the PSUM->SBUF copy (ACT Identity bias).
Cross-core coupling is two small AllReduces: the uni-branch sum-of-squares
(hidden under streaming; its result is only read after the last stream so
the sync DMA queue never head-of-line blocks) and the 8KB out-norm
partials on the tail (the t*z fold runs during its flight).

kernel(**inputs) takes the full unsharded inputs and returns the full
[1024, 1536] f32 output. Host-side prep does layout/dtype staging only --
all arithmetic runs on device.
"""

import numpy as np
import ml_dtypes

import concourse.bass as bass
import concourse.tile as tile
import concourse.mybir as mybir
from concourse import bacc
from concourse.bass_utils import run_bass_kernel_spmd

f32 = mybir.dt.float32
f32r = mybir.dt.float32r
bf16 = mybir.dt.bfloat16
AX = mybir.AxisListType
ALU = mybir.AluOpType
ACTF = mybir.ActivationFunctionType

NCORES = 8
B, N, G, K, D = 1024, 10, 7, 1024, 1536
DS = D // NCORES          # 192 output dims per core
KT = K // 128             # 8 k-tiles
KH = KT // 2              # 4 k-tiles per W half-transfer
BSL = 512                 # free-dim slice for matmuls
NBS = B // BSL            # 2
ROUTER_GS = (0, 4, 8)     # TEXT_PRED groups, streamed first
STREAM_GS = (0, 4, 8, 1, 2, 3, 5, 6, 7, 9)
CHUNKS = ((0, 128), (128, 64))  # (d-offset, size) chunks of DS=192
EPS_BN = 1e-5
EPS_NORM = 1e-12

LAST_RESULTS = None
_NC_CACHE = {}


def _emit(nc, tc):
    # ---- DRAM I/O ----
    xT_d = nc.dram_tensor("xT", [N, 128, KT, B], bf16, kind="ExternalInput").ap()
    xibT_d = nc.dram_tensor("xibT", [128, KT, B], bf16, kind="ExternalInput").ap()
    w_d = nc.dram_tensor("w", [N, 128, KT, G, DS], bf16, kind="ExternalInput").ap()
    wib_d = nc.dram_tensor("wib", [128, KT, G, DS], bf16, kind="ExternalInput").ap()
    xuT_d = nc.dram_tensor("xuT", [DS, B], bf16, kind="ExternalInput").ap()
    bp_d = nc.dram_tensor("bp", [N, DS, G], bf16, kind="ExternalInput").ap()
    bib_d = nc.dram_tensor("bib", [DS, G], bf16, kind="ExternalInput").ap()
    rw1_d = nc.dram_tensor("rw1", [3, 128, KT, BSL], bf16, kind="ExternalInput").ap()
    rw2_d = nc.dram_tensor("rw2", [128, 4, 100], bf16, kind="ExternalInput").ap()
    rw3_d = nc.dram_tensor("rw3", [100, 11], bf16, kind="ExternalInput").ap()
    rb3_d = nc.dram_tensor("rb3", [11, 1], f32, kind="ExternalInput").ap()
    sel_d = nc.dram_tensor("sel", [11, 11, 128], f32r, kind="ExternalInput").ap()
    ones_d = nc.dram_tensor("onesd", [128, 130], f32r, kind="ExternalInput").ap()
    outT_d = nc.dram_tensor("outT", [DS, B], f32, kind="ExternalOutput").ap()

    pools = {}

    def pool(name, bufs, space="SBUF"):
        cm = tc.tile_pool(name=name, bufs=bufs, space=space)
        pools[name] = cm
        return cm.__enter__()

    cp = pool("const", 1)       # persistent constants / accumulators
    xtp = pool("xt", 2)         # [128, KT, B] bf16 whole-group x
    wtp = pool("wt", 2)         # [128, KH, G, DS] bf16 half-group W
    wsp = pool("ws", 2)         # [128, KT, DS] bf16 e-reduced weights
    wrp = pool("wred", 3)       # [128, KH, DS] bf16 e-reduce tree temps
    gs0 = pool("gs0", 6)        # [128, B] bf16 staged raw (chunk 0)
    gs1 = pool("gs1", 6)        # [64, B] bf16 staged raw (chunk 1)
    rwp = pool("rwt", 2)        # [128, KT, BSL] bf16 rw1 per router group
    bpp = pool("bpg", 3)        # [128, G] bf16 per-group bias staging
    bsp = pool("bsum", 6)       # [128, 1] f32 e-reduced bias columns
    b1k = pool("big1k", 2)      # [128, B] scratch
    ctp = pool("ctmp", 3)       # [128, BSL] scratch
    stp = pool("stat", 10)      # [128, 1] BN stats smalls
    psp = pool("ps", 8, space="PSUM")
    drp = pool("dram", 1, space="DRAM")

    def ps_tile(p, n=BSL):
        return psp.tile([p, n], f32, tag="ps", name="ps")

    # ---- constants ----
    ones1 = cp.tile([1, 128], f32r, tag="ones1", name="ones1")
    nc.gpsimd.dma_start(ones1[:], ones_d[0:1, 2:130])
    onesb = cp.tile([128, 1], f32r, tag="onesb", name="onesb")
    nc.gpsimd.dma_start(onesb[:], ones_d[:, 0:1])
    twosb = cp.tile([128, 1], f32r, tag="twosb", name="twosb")
    nc.gpsimd.dma_start(twosb[:], ones_d[:, 1:2])
    sel = cp.tile([11, 11, 128], f32r, tag="sel", name="sel")
    nc.gpsimd.dma_start(sel[:], sel_d[:])
    rb3 = cp.tile([11, 1], f32, tag="rb3", name="rb3")
    nc.gpsimd.dma_start(rb3[:], rb3_d[:])
    rw2 = cp.tile([128, 4, 100], bf16, tag="rw2", name="rw2")
    nc.gpsimd.dma_start(rw2[:], rw2_d[:])
    rw3 = cp.tile([100, 11], bf16, tag="rw3", name="rw3")
    nc.gpsimd.dma_start(rw3[:], rw3_d[:])
    xu = []
    for ci, (m0, msz) in enumerate(CHUNKS):
        t = cp.tile([msz, B], bf16, tag=f"xu{ci}", name=f"xu{ci}")
        nc.sync.dma_start(t[:], xuT_d[m0:m0 + msz, :])
        xu.append(t)

    # ---- persistent buffers ----
    h1 = [cp.tile([128, B], f32, tag=f"h1_{m}", name=f"h1_{m}") for m in range(4)]
    h1b = [cp.tile([128, B], bf16, tag=f"h1b_{m}", name=f"h1b_{m}") for m in range(4)]
    A = [cp.tile([msz, B], f32, tag=f"A{ci}", name=f"A{ci}") for ci, (m0, msz) in enumerate(CHUNKS)]
    z = [cp.tile([msz, B], f32, tag=f"z{ci}", name=f"z{ci}") for ci, (m0, msz) in enumerate(CHUNKS)]
    outsb = [cp.tile([msz, B], f32, tag=f"out{ci}", name=f"out{ci}") for ci, (m0, msz) in enumerate(CHUNKS)]
    h2sb = cp.tile([100, B], f32, tag="h2sb", name="h2sb")
    h2b = cp.tile([100, B], bf16, tag="h2b", name="h2b")
    ex10 = cp.tile([11, B], f32r, tag="ex10", name="ex10")
    arv = cp.tile([1, 3 * B], f32, tag="arv", name="arv")
    tv = cp.tile([1, B], f32r, tag="tv", name="tv")
    uv = cp.tile([1, B], f32r, tag="uv", name="uv")

    cc1_in = drp.tile([1, B], f32, tag="cc1_in", name="cc1_in")
    cc1_out = drp.tile([1, B], f32, tag="cc1_out", addr_space="Shared", name="cc1_out")
    cc2_in = drp.tile([1, B], f32, tag="cc2_in", name="cc2_in")
    cc2_out = drp.tile([1, B], f32, tag="cc2_out", addr_space="Shared", name="cc2_out")

    for ci in range(len(CHUNKS)):
        nc.vector.memset(A[ci][:], 0.0)

    def load_bias_cols(bsrc):
        """Load [DS, G] bias, e-reduce to per-chunk [msz, 1] f32 columns."""
        cols = []
        for ci, (m0, msz) in enumerate(CHUNKS):
            bc_t = bpp.tile([128, G], bf16, tag="bpg", name="bpg")
            nc.gpsimd.dma_start(bc_t[0:msz, :], bsrc[m0:m0 + msz, :])
            col = bsp.tile([128, 1], f32, tag="bsum", name="bsum")
            nc.vector.reduce_sum(out=col[0:msz, :], in_=bc_t[0:msz, :], axis=AX.X)
            cols.append(col)
        return cols

    def stream_group(xsrc, wsrc):
        """DMA one group's x (one shot) + W (two halves), reduce W over e.

        The e-reduction is a pairwise add tree with expert slices contiguous
        along DS (step-1 bf16 operands hit the DVE 2x mode; DVE's 1x
        tensor_reduce over a stride-7 axis measured 2x slower). The first
        tree level is offloaded to GpSimd, which is otherwise idle.
        """
        xt = xtp.tile([128, KT, B], bf16, tag="xt", name="xt")
        nc.sync.dma_start(xt[:], xsrc)
        ws = wsp.tile([128, KT, DS], bf16, tag="ws", name="ws")
        for h in range(2):
            wt = wtp.tile([128, KH, G, DS], bf16, tag="wt", name="wt")
            nc.sync.dma_start(wt[:], wsrc[:, h * KH:(h + 1) * KH, :, :])
            t01 = wrp.tile([128, KH, DS], bf16, tag="wred", name="wred")
            nc.vector.tensor_tensor(out=t01[:], in0=wt[:, :, 0, :],
                                    in1=wt[:, :, 1, :], op=ALU.add)
            t23 = wrp.tile([128, KH, DS], bf16, tag="wred", name="wred")
            nc.vector.tensor_tensor(out=t23[:], in0=wt[:, :, 2, :],
                                    in1=wt[:, :, 3, :], op=ALU.add)
            t45 = wrp.tile([128, KH, DS], bf16, tag="wred", name="wred")
            nc.vector.tensor_tensor(out=t45[:], in0=wt[:, :, 4, :],
                                    in1=wt[:, :, 5, :], op=ALU.add)
            nc.vector.tensor_tensor(out=t01[:], in0=t01[:], in1=t23[:],
                                    op=ALU.add)
            nc.vector.tensor_tensor(out=t45[:], in0=t45[:], in1=wt[:, :, 6, :],
                                    op=ALU.add)
            nc.vector.tensor_tensor(out=ws[:, h * KH:(h + 1) * KH, :],
                                    in0=t01[:], in1=t45[:], op=ALU.add)
        return xt, ws

    def group_matmuls(xt, ws, bias_cols):
        """raw = x @ Wsum (+ bias on the ACT copy); stage to SBUF bf16."""
        raws = []
        for ci, (m0, msz) in enumerate(CHUNKS):
            gp = gs0 if ci == 0 else gs1
            raw = gp.tile([msz, B], bf16, tag=f"gs{ci}", name=f"gs{ci}")
            for bs in range(NBS):
                sl = slice(bs * BSL, (bs + 1) * BSL)
                ps = ps_tile(msz)
                for kt in range(KT):
                    nc.tensor.matmul(
                        ps[:],
                        lhsT=ws[:, kt, m0:m0 + msz],
                        rhs=xt[:, kt, sl],
                        start=(kt == 0), stop=(kt == KT - 1))
                nc.scalar.activation(raw[:, sl], ps[:], ACTF.Identity,
                                     bias=bias_cols[ci][0:msz, :])
            raws.append(raw)
        return raws

    def fold_group(raws, i):
        """A += e_i * raw (e broadcast across partitions via sel matmul)."""
        for bs in range(NBS):
            sl = slice(bs * BSL, (bs + 1) * BSL)
            bc = ps_tile(128)
            nc.tensor.matmul(bc[:], lhsT=sel[:, i, :],
                             rhs=ex10[:, sl], start=True, stop=True)
            for ci, (m0, msz) in enumerate(CHUNKS):
                tmp = ctp.tile([128, BSL], f32, tag="ctmp", name="ctmp")
                nc.vector.tensor_tensor(out=tmp[0:msz, :], in0=raws[ci][:, sl],
                                        in1=bc[0:msz, :], op=ALU.mult)
                nc.vector.tensor_tensor(out=A[ci][:, sl], in0=A[ci][:, sl],
                                        in1=tmp[0:msz, :], op=ALU.add)

    # ================= uni branch first -> z, ssz partial, AR1 =========
    xib_t, wib_s = stream_group(xibT_d, wib_d)
    bib_cols = load_bias_cols(bib_d)
    bib_sc = []
    for ci, (m0, msz) in enumerate(CHUNKS):
        c2 = bsp.tile([128, 1], f32, tag="bsum", name="bsum")
        nc.vector.tensor_scalar_mul(c2[0:msz, :], bib_cols[ci][0:msz, :], 0.1 / 7.0)
        bib_sc.append(c2)
    for ci, (m0, msz) in enumerate(CHUNKS):
        for bs in range(NBS):
            sl = slice(bs * BSL, (bs + 1) * BSL)
            ps = ps_tile(msz)
            for kt in range(KT):
                nc.tensor.matmul(ps[:], lhsT=wib_s[:, kt, m0:m0 + msz],
                                 rhs=xib_t[:, kt, sl],
                                 start=(kt == 0), stop=(kt == KT - 1))
            # z = (raw + bias)*0.1/7 + 0.9*xu
            nc.scalar.activation(z[ci][:, sl], ps[:], ACTF.Identity,
                                 scale=0.1 / 7.0, bias=bib_sc[ci][0:msz, :])
            t9 = ctp.tile([128, BSL], f32, tag="ctmp", name="ctmp")
            nc.vector.tensor_scalar_mul(t9[0:msz, :], xu[ci][:, sl], 0.9)
            nc.vector.tensor_tensor(out=z[ci][:, sl], in0=z[ci][:, sl],
                                    in1=t9[0:msz, :], op=ALU.add)
    # ssz partial: sum_d z^2 -> arv slot 2 -> cc1_in
    for bs in range(NBS):
        sl = slice(bs * BSL, (bs + 1) * BSL)
        ps = ps_tile(1)
        for ci, (m0, msz) in enumerate(CHUNKS):
            sq = b1k.tile([128, B], f32r, tag="big1k", name="big1k")
            nc.scalar.square(sq[0:msz, 0:BSL], z[ci][:, sl])
            nc.tensor.matmul(ps[:], lhsT=onesb[0:msz, :],
                             rhs=sq[0:msz, 0:BSL],
                             start=(ci == 0), stop=(ci == len(CHUNKS) - 1))
        nc.scalar.copy(arv[:, B + bs * BSL:B + (bs + 1) * BSL], ps[:])
    nc.sync.dma_start(cc1_in[:], arv[:, B:2 * B])

    # ================= router groups (also feed h1) =================
    router_raws = {}
    for ri, g in enumerate(ROUTER_GS):
        xt, ws = stream_group(xT_d[g], w_d[g])
        bcols = load_bias_cols(bp_d[g])
        router_raws[g] = group_matmuls(xt, ws, bcols)
        if ri == 0:
            nc.gpsimd.collective_compute(
                "AllReduce", ALU.add,
                ins=[cc1_in.opt()], outs=[cc1_out.opt()],
                replica_groups=[list(range(NCORES))])
        # router h1 partial: h1[m] (+)= rw1[g-block].T @ xT[g]
        rt = rwp.tile([128, KT, BSL], bf16, tag="rwt", name="rwt")
        nc.sync.dma_start(rt[:], rw1_d[ri])
        for m in range(4):
            for bs in range(NBS):
                sl = slice(bs * BSL, (bs + 1) * BSL)
                ps = ps_tile(128)
                for kt in range(KT):
                    nc.tensor.matmul(
                        ps[:],
                        lhsT=rt[:, kt, m * 128:(m + 1) * 128],
                        rhs=xt[:, kt, sl],
                        start=(kt == 0), stop=(kt == KT - 1))
                if ri == 0:
                    nc.scalar.copy(h1[m][:, sl], ps[:])
                else:
                    nc.vector.tensor_tensor(out=h1[m][:, sl], in0=ps[:],
                                            in1=h1[m][:, sl], op=ALU.add)

    # ======= stream group 1 before the finalize (overlap its DMA/PE) ===
    g1 = STREAM_GS[3]
    xt1, ws1 = stream_group(xT_d[g1], w_d[g1])
    bcols1 = load_bias_cols(bp_d[g1])
    raws1 = group_matmuls(xt1, ws1, bcols1)

    # ================= router finalize =================
    def bn_act(tiles, out_tiles, nparts, func):
        """BatchNorm (training stats over free axis) + activation."""
        for t, to in zip(tiles, out_tiles):
            dump = b1k.tile([128, B], f32, tag="big1k", name="big1k")
            mnr = stp.tile([128, 1], f32, tag="stat", name="stat")
            nc.scalar.activation(dump[0:nparts, :], t[:], ACTF.Copy,
                                 accum_out=mnr[0:nparts, :])
            mn = stp.tile([128, 1], f32, tag="stat", name="stat")
            nc.scalar.mul(mn[0:nparts, :], mnr[0:nparts, :], 1.0 / B)
            sq = b1k.tile([128, B], f32, tag="big1k", name="big1k")
            ex2r = stp.tile([128, 1], f32, tag="stat", name="stat")
            nc.scalar.activation(sq[0:nparts, :], t[:], ACTF.Square,
                                 accum_out=ex2r[0:nparts, :])
            ex2 = stp.tile([128, 1], f32, tag="stat", name="stat")
            nc.scalar.mul(ex2[0:nparts, :], ex2r[0:nparts, :], 1.0 / B)
            var = stp.tile([128, 1], f32, tag="stat", name="stat")
            nc.vector.tensor_tensor(out=var[0:nparts, :], in0=mn[0:nparts, :],
                                    in1=mn[0:nparts, :], op=ALU.mult)
            nc.vector.tensor_tensor(out=var[0:nparts, :], in0=ex2[0:nparts, :],
                                    in1=var[0:nparts, :], op=ALU.subtract)
            nc.vector.tensor_scalar_add(var[0:nparts, :], var[0:nparts, :], EPS_BN)
            sd = stp.tile([128, 1], f32, tag="stat", name="stat")
            nc.scalar.sqrt(sd[0:nparts, :], var[0:nparts, :])
            rs = stp.tile([128, 1], f32, tag="stat", name="stat")
            nc.vector.reciprocal(rs[0:nparts, :], sd[0:nparts, :])
            nb = stp.tile([128, 1], f32, tag="stat", name="stat")
            nc.vector.tensor_tensor(out=nb[0:nparts, :], in0=mn[0:nparts, :],
                                    in1=rs[0:nparts, :], op=ALU.mult)
            nc.vector.tensor_scalar_mul(nb[0:nparts, :], nb[0:nparts, :], -1.0)
            nc.scalar.activation(to[:], t[:], func,
                                 bias=nb[0:nparts, :], scale=rs[0:nparts, :])

    bn_act(h1, h1b, 128, ACTF.Relu)
    for bs in range(NBS):
        sl = slice(bs * BSL, (bs + 1) * BSL)
        ps = ps_tile(100)
        for kt in range(4):
            nc.tensor.matmul(ps[:], lhsT=rw2[:, kt, :],
                             rhs=h1b[kt][:, sl],
                             start=(kt == 0), stop=(kt == 3))
        nc.scalar.copy(h2sb[:, sl], ps[:])
    bn_act([h2sb], [h2b], 100, ACTF.Tanh)
    for bs in range(NBS):
        sl = slice(bs * BSL, (bs + 1) * BSL)
        ps = ps_tile(11)
        nc.tensor.matmul(ps[:], lhsT=rw3[:],
                         rhs=h2b[:, sl], start=True, stop=True)
        sg = ctp.tile([128, BSL], f32, tag="ctmp", name="ctmp")
        nc.scalar.activation(sg[0:11, :], ps[:], ACTF.Sigmoid, bias=rb3[:], scale=1.0)
        nc.scalar.activation(ex10[:, sl], sg[0:11, :], ACTF.Exp, scale=10.0)

    # e10 row (no AR1 dependency): arv slot 3 <- 7*e10, then (7*e10)^2/8
    for bs in range(NBS):
        sl = slice(bs * BSL, (bs + 1) * BSL)
        bc = ps_tile(128)
        nc.tensor.matmul(bc[:], lhsT=sel[:, 10, :],
                         rhs=ex10[:, sl], start=True, stop=True)
        nc.scalar.copy(arv[:, 2 * B + bs * BSL:2 * B + (bs + 1) * BSL], bc[0:1, :])
    e10v = arv[:, 2 * B:3 * B]
    nc.vector.tensor_scalar_mul(e10v, e10v, 7.0)

    # ================= folds: router groups + g1, then stream rest =====
    for g in STREAM_GS[:3]:
        fold_group(router_raws[g], g)
    fold_group(raws1, g1)

    for gi, g in enumerate(STREAM_GS[4:]):
        xt, ws = stream_group(xT_d[g], w_d[g])
        bcols = load_bias_cols(bp_d[g])
        raws = group_matmuls(xt, ws, bcols)
        fold_group(raws, g)
        if gi == 1:
            # AR1 result fetched mid-kernel on the gpsimd queue (AR1 is done
            # by now; the remaining gpsimd work behind it isn't needed until
            # much later, so the wait can't stall anything that matters).
            nc.gpsimd.dma_start(arv[:, B:2 * B], cc1_out[:])
            s_v = arv[:, B:2 * B]
            nc.scalar.sqrt(s_v, s_v)
            nc.vector.tensor_scalar_max(s_v, s_v, EPS_NORM)
            nc.vector.reciprocal(uv[:], s_v)
            nc.vector.tensor_tensor(out=tv[:], in0=e10v, in1=uv[:], op=ALU.mult)
            # e10sq/8: each core contributes 1/8 so the AR2 sum restores it
            nc.vector.tensor_tensor(out=e10v, in0=e10v, in1=e10v, op=ALU.mult)
            nc.vector.tensor_scalar_mul(e10v, e10v, 0.125)

    # ========= tail: q_loc = |A|^2 + t*(2 A.z) + (7e10)^2/8 -> AR2 ======
    for bs in range(NBS):
        sl = slice(bs * BSL, (bs + 1) * BSL)
        psa = ps_tile(1)
        psc = ps_tile(1)
        for ci, (m0, msz) in enumerate(CHUNKS):
            sqa = b1k.tile([128, B], f32r, tag="big1k", name="big1k")
            nc.scalar.square(sqa[0:msz, 0:BSL], A[ci][:, sl])
            nc.tensor.matmul(psa[:], lhsT=onesb[0:msz, :],
                             rhs=sqa[0:msz, 0:BSL],
                             start=(ci == 0), stop=(ci == len(CHUNKS) - 1))
            cza = b1k.tile([128, B], f32r, tag="big1k", name="big1k")
            nc.vector.tensor_tensor(out=cza[0:msz, 0:BSL], in0=A[ci][:, sl],
                                    in1=z[ci][:, sl], op=ALU.mult)
            nc.tensor.matmul(psc[:], lhsT=twosb[0:msz, :],
                             rhs=cza[0:msz, 0:BSL],
                             start=(ci == 0), stop=(ci == len(CHUNKS) - 1))
        qt = ctp.tile([128, BSL], f32, tag="ctmp", name="ctmp")
        nc.vector.tensor_tensor(out=qt[0:1, :], in0=psc[:], in1=tv[:, sl],
                                op=ALU.mult)
        nc.vector.tensor_tensor(out=qt[0:1, :], in0=qt[0:1, :], in1=psa[:],
                                op=ALU.add)
        nc.vector.tensor_tensor(out=arv[:, bs * BSL:(bs + 1) * BSL],
                                in0=qt[0:1, :],
                                in1=arv[:, 2 * B + bs * BSL:2 * B + (bs + 1) * BSL],
                                op=ALU.add)
    nc.sync.dma_start(cc2_in[:], arv[:, 0:B])
    nc.gpsimd.collective_compute(
        "AllReduce", ALU.add,
        ins=[cc2_in.opt()], outs=[cc2_out.opt()],
        replica_groups=[list(range(NCORES))])

    # P = A + t*z, computed while AR2 is in flight
    for bs in range(NBS):
        sl = slice(bs * BSL, (bs + 1) * BSL)
        btv = ps_tile(128)
        nc.tensor.matmul(btv[:], lhsT=ones1[:],
                         rhs=tv[:, sl], start=True, stop=True)
        for ci, (m0, msz) in enumerate(CHUNKS):
            t2 = ctp.tile([128, BSL], f32, tag="ctmp", name="ctmp")
            nc.vector.tensor_tensor(out=t2[0:msz, :], in0=z[ci][:, sl],
                                    in1=btv[0:msz, :], op=ALU.mult)
            nc.vector.tensor_tensor(out=A[ci][:, sl], in0=A[ci][:, sl],
                                    in1=t2[0:msz, :], op=ALU.add)

    nc.sync.dma_start(arv[:, 0:B], cc2_out[:])

    # q -> u = 1/max(sqrt(q), eps); out = P*u
    a_v = arv[:, 0:B]
    nc.scalar.sqrt(a_v, a_v)
    nc.vector.tensor_scalar_max(a_v, a_v, EPS_NORM)
    nc.vector.reciprocal(uv[:], a_v)
    for bs in range(NBS):
        sl = slice(bs * BSL, (bs + 1) * BSL)
        bu = ps_tile(128)
        nc.tensor.matmul(bu[:], lhsT=ones1[:],
                         rhs=uv[:, sl], start=True, stop=True)
        for ci, (m0, msz) in enumerate(CHUNKS):
            nc.vector.tensor_tensor(out=outsb[ci][:, sl], in0=A[ci][:, sl],
                                    in1=bu[0:msz, :], op=ALU.mult)
    for ci, (m0, msz) in enumerate(CHUNKS):
        nc.sync.dma_start(outT_d[m0:m0 + msz, :], outsb[ci][:])

    for p in reversed(list(pools.values())):
        p.__exit__(None, None, None)


def _build_nc():
    nc = bacc.Bacc("TRN2", target_bir_lowering=False, debug=False,
                   num_devices=NCORES)
    with tile.TileContext(nc) as tc:
        with nc.allow_low_precision(reason="bf16 streams / f32r reductions are intentional"):
            _emit(nc, tc)
    nc.compile()
    return nc


def _as_bf16(a):
    return np.ascontiguousarray(a.astype(ml_dtypes.bfloat16))


def _host_prep(inputs):
    x_enc = np.asarray(inputs["x_enc"], dtype=np.float32)
    x_ib = np.asarray(inputs["x_ib"], dtype=np.float32)
    x_uni = np.asarray(inputs["x_uni"], dtype=np.float32)
    W_proj = np.asarray(inputs["W_proj"], dtype=np.float32)
    b_proj = np.asarray(inputs["b_proj"], dtype=np.float32)
    W_ib = np.asarray(inputs["W_ib"], dtype=np.float32)
    b_ib = np.asarray(inputs["b_ib"], dtype=np.float32)

    # x_enc [N,B,K] -> [N, 128, KT, B] partition-major bf16
    xT = _as_bf16(x_enc.transpose(0, 2, 1).reshape(N, KT, 128, B).transpose(0, 2, 1, 3))
    # x_ib [B,K] -> [128, KT, B]
    xibT = _as_bf16(x_ib.T.reshape(KT, 128, B).transpose(1, 0, 2))
    sel = np.zeros((11, 11, 128), dtype=np.float32)
    for q in range(11):
        sel[q, q, :] = 1.0
    rb3 = np.ascontiguousarray(np.asarray(inputs["r_b3"], np.float32).reshape(11, 1))
    # r_w1 [3072, 512] -> [3, 128, KT, 512]
    rw1 = _as_bf16(np.asarray(inputs["r_w1"], np.float32)
                   .reshape(3, KT, 128, BSL).transpose(0, 2, 1, 3))
    rw2 = _as_bf16(np.asarray(inputs["r_w2"], np.float32)
                   .reshape(4, 128, 100).transpose(1, 0, 2))
    rw3 = _as_bf16(np.asarray(inputs["r_w3"], np.float32))
    ones_host = np.ones((128, 130), dtype=np.float32)
    ones_host[:, 1] = 2.0

    in_maps = []
    for c in range(NCORES):
        ds = slice(c * DS, (c + 1) * DS)
        # W_proj [N,G,K,D] ds-slice -> [N, 128, KT, G, DS]
        wc = _as_bf16(W_proj[:, :, :, ds].reshape(N, G, KT, 128, DS)
                      .transpose(0, 3, 2, 1, 4))
        wibc = _as_bf16(W_ib[:, :, ds].reshape(G, KT, 128, DS)
                        .transpose(2, 1, 0, 3))
        in_maps.append({
            "xT": xT,
            "xibT": xibT,
            "w": wc,
            "wib": wibc,
            "xuT": _as_bf16(x_uni[:, ds].T),
            "bp": _as_bf16(b_proj[:, :, ds].transpose(0, 2, 1)),
            "bib": _as_bf16(b_ib[:, ds].T),
            "rw1": rw1,
            "rw2": rw2,
            "rw3": rw3,
            "rb3": rb3,
            "sel": sel,
            "onesd": ones_host,
        })
    return in_maps


def kernel(**inputs):
    global LAST_RESULTS
    if "nc" not in _NC_CACHE:
        _NC_CACHE["nc"] = _build_nc()
    nc = _NC_CACHE["nc"]
    in_maps = _host_prep(inputs)
    res = run_bass_kernel_spmd(nc, in_maps, list(range(NCORES)))
    LAST_RESULTS = res
    full = np.concatenate([res.results[c]["outT"] for c in range(NCORES)], axis=0)
    return np.ascontiguousarray(full.T)


# revision 21
# speedup vs baseline: 1.1186x; 1.1186x over previous
"""Trainium2 Bass kernel for nn_ATVP_router_UNI (moe_routing).

Sharding: output dim D=1536 sharded over 8 cores (192 each). Activations
(x_enc, x_ib) are broadcast in a partition-major transposed layout; the
router MLP is replicated on every core. The expert mean over e commutes
with the linear projection, so each core streams its W_proj slice once
(bf16), reduces over e (split across GpSimd and DVE), and runs 1/7 of the
naive matmul FLOPs in bf16. All heavy streams are cast to bf16 on the
host -- this halves HBM traffic, which is the roofline for this kernel.
The softmax denominator and the 1/7 group-mean scale cancel inside the
final l2 normalization and are folded away. Per-expert biases are
e-reduced on DVE and folded into the PSUM->SBUF copy (ACT Identity bias).
Cross-core coupling is two small AllReduces: the uni-branch sum-of-squares
(hidden under streaming; its result is only read after the last stream so
the sync DMA queue never head-of-line blocks) and the 8KB out-norm
partials on the tail (the t*z fold runs during its flight).

kernel(**inputs) takes the full unsharded inputs and returns the full
[1024, 1536] f32 output. Host-side prep does layout/dtype staging only --
all arithmetic runs on device.
"""

import numpy as np
import ml_dtypes

import concourse.bass as bass
import concourse.tile as tile
import concourse.mybir as mybir
from concourse import bacc
from concourse.bass_utils import run_bass_kernel_spmd

f32 = mybir.dt.float32
f32r = mybir.dt.float32r
bf16 = mybir.dt.bfloat16
AX = mybir.AxisListType
ALU = mybir.AluOpType
ACTF = mybir.ActivationFunctionType

NCORES = 8
B, N, G, K, D = 1024, 10, 7, 1024, 1536
DS = D // NCORES          # 192 output dims per core
KT = K // 128             # 8 k-tiles
KH = KT // 2              # 4 k-tiles per W half-transfer
BSL = 512                 # free-dim slice for matmuls
NBS = B // BSL            # 2
ROUTER_GS = (0, 4, 8)     # TEXT_PRED groups, streamed first
STREAM_GS = (0, 4, 8, 1, 2, 3, 5, 6, 7, 9)
CHUNKS = ((0, 128), (128, 64))  # (d-offset, size) chunks of DS=192
EPS_BN = 1e-5
EPS_NORM = 1e-12

LAST_RESULTS = None
_NC_CACHE = {}


def _emit(nc, tc):
    # ---- DRAM I/O ----
    xT_d = nc.dram_tensor("xT", [N, 128, KT, B], bf16, kind="ExternalInput").ap()
    xibT_d = nc.dram_tensor("xibT", [128, KT, B], bf16, kind="ExternalInput").ap()
    w_d = nc.dram_tensor("w", [N, 128, KT, G, DS], bf16, kind="ExternalInput").ap()
    wib_d = nc.dram_tensor("wib", [128, KT, G, DS], bf16, kind="ExternalInput").ap()
    xuT_d = nc.dram_tensor("xuT", [DS, B], bf16, kind="ExternalInput").ap()
    bp_d = nc.dram_tensor("bp", [N, DS, G], bf16, kind="ExternalInput").ap()
    bib_d = nc.dram_tensor("bib", [DS, G], bf16, kind="ExternalInput").ap()
    rw1_d = nc.dram_tensor("rw1", [3, 128, KT, BSL], bf16, kind="ExternalInput").ap()
    rw2_d = nc.dram_tensor("rw2", [128, 4, 100], bf16, kind="ExternalInput").ap()
    rw3_d = nc.dram_tensor("rw3", [100, 11], bf16, kind="ExternalInput").ap()
    rb3_d = nc.dram_tensor("rb3", [11, 1], f32, kind="ExternalInput").ap()
    sel_d = nc.dram_tensor("sel", [11, 11, 128], f32r, kind="ExternalInput").ap()
    ones_d = nc.dram_tensor("onesd", [128, 130], f32r, kind="ExternalInput").ap()
    outT_d = nc.dram_tensor("outT", [DS, B], f32, kind="ExternalOutput").ap()

    pools = {}

    def pool(name, bufs, space="SBUF"):
        cm = tc.tile_pool(name=name, bufs=bufs, space=space)
        pools[name] = cm
        return cm.__enter__()

    cp = pool("const", 1)       # persistent constants / accumulators
    xtp = pool("xt", 2)         # [128, KT, B] bf16 whole-group x
    wtp = pool("wt", 2)         # [128, KH, G, DS] bf16 half-group W
    wsp = pool("ws", 2)         # [128, KT, DS] bf16 e-reduced weights
    wrp = pool("wred", 3)       # [128, KH, DS] bf16 e-reduce tree temps
    gs0 = pool("gs0", 6)        # [128, B] bf16 staged raw (chunk 0)
    gs1 = pool("gs1", 6)        # [64, B] bf16 staged raw (chunk 1)
    rwp = pool("rwt", 2)        # [128, KT, BSL] bf16 rw1 per router group
    bpp = pool("bpg", 3)        # [128, G] bf16 per-group bias staging
    bsp = pool("bsum", 6)       # [128, 1] f32 e-reduced bias columns
    b1k = pool("big1k", 2)      # [128, B] scratch
    ctp = pool("ctmp", 3)       # [128, BSL] scratch
    stp = pool("stat", 10)      # [128, 1] BN stats smalls
    psp = pool("ps", 8, space="PSUM")
    drp = pool("dram", 1, space="DRAM")

    def ps_tile(p, n=BSL):
        return psp.tile([p, n], f32, tag="ps", name="ps")

    # ---- constants ----
    ones1 = cp.tile([1, 128], f32r, tag="ones1", name="ones1")
    nc.gpsimd.dma_start(ones1[:], ones_d[0:1, 2:130])
    onesb = cp.tile([128, 1], f32r, tag="onesb", name="onesb")
    nc.gpsimd.dma_start(onesb[:], ones_d[:, 0:1])
    twosb = cp.tile([128, 1], f32r, tag="twosb", name="twosb")
    nc.gpsimd.dma_start(twosb[:], ones_d[:, 1:2])
    sel = cp.tile([11, 11, 128], f32r, tag="sel", name="sel")
    nc.gpsimd.dma_start(sel[:], sel_d[:])
    rb3 = cp.tile([11, 1], f32, tag="rb3", name="rb3")
    nc.gpsimd.dma_start(rb3[:], rb3_d[:])
    rw2 = cp.tile([128, 4, 100], bf16, tag="rw2", name="rw2")
    nc.gpsimd.dma_start(rw2[:], rw2_d[:])
    rw3 = cp.tile([100, 11], bf16, tag="rw3", name="rw3")
    nc.gpsimd.dma_start(rw3[:], rw3_d[:])
    xu = []
    for ci, (m0, msz) in enumerate(CHUNKS):
        t = cp.tile([msz, B], bf16, tag=f"xu{ci}", name=f"xu{ci}")
        nc.sync.dma_start(t[:], xuT_d[m0:m0 + msz, :])
        xu.append(t)

    # ---- persistent buffers ----
    h1 = [cp.tile([128, B], f32, tag=f"h1_{m}", name=f"h1_{m}") for m in range(4)]
    h1b = [cp.tile([128, B], bf16, tag=f"h1b_{m}", name=f"h1b_{m}") for m in range(4)]
    A = [cp.tile([msz, B], f32, tag=f"A{ci}", name=f"A{ci}") for ci, (m0, msz) in enumerate(CHUNKS)]
    z = [cp.tile([msz, B], f32, tag=f"z{ci}", name=f"z{ci}") for ci, (m0, msz) in enumerate(CHUNKS)]
    outsb = [cp.tile([msz, B], f32, tag=f"out{ci}", name=f"out{ci}") for ci, (m0, msz) in enumerate(CHUNKS)]
    h2sb = cp.tile([100, B], f32, tag="h2sb", name="h2sb")
    h2b = cp.tile([100, B], bf16, tag="h2b", name="h2b")
    ex10 = cp.tile([11, B], f32r, tag="ex10", name="ex10")
    arv = cp.tile([1, 3 * B], f32, tag="arv", name="arv")
    tv = cp.tile([1, B], f32r, tag="tv", name="tv")
    uv = cp.tile([1, B], f32r, tag="uv", name="uv")

    cc1_in = drp.tile([1, B], f32, tag="cc1_in", name="cc1_in")
    cc1_out = drp.tile([1, B], f32, tag="cc1_out", addr_space="Shared", name="cc1_out")
    cc2_in = drp.tile([1, B], f32, tag="cc2_in", name="cc2_in")
    cc2_out = drp.tile([1, B], f32, tag="cc2_out", addr_space="Shared", name="cc2_out")

    for ci in range(len(CHUNKS)):
        nc.vector.memset(A[ci][:], 0.0)

    def load_bias_cols(bsrc):
        """Load [DS, G] bias, e-reduce to per-chunk [msz, 1] f32 columns."""
        cols = []
        for ci, (m0, msz) in enumerate(CHUNKS):
            bc_t = bpp.tile([128, G], bf16, tag="bpg", name="bpg")
            nc.gpsimd.dma_start(bc_t[0:msz, :], bsrc[m0:m0 + msz, :])
            col = bsp.tile([128, 1], f32, tag="bsum", name="bsum")
            nc.vector.reduce_sum(out=col[0:msz, :], in_=bc_t[0:msz, :], axis=AX.X)
            cols.append(col)
        return cols

    def stream_group(xsrc, wsrc):
        """DMA one group's x (one shot) + W (two halves), reduce W over e.

        The e-reduction is a pairwise add tree with expert slices contiguous
        along DS (step-1 bf16 operands hit the DVE 2x mode; DVE's 1x
        tensor_reduce over a stride-7 axis measured 2x slower). The first
        tree level is offloaded to GpSimd, which is otherwise idle.
        """
        xt = xtp.tile([128, KT, B], bf16, tag="xt", name="xt")
        nc.sync.dma_start(xt[:], xsrc)
        ws = wsp.tile([128, KT, DS], bf16, tag="ws", name="ws")
        for h in range(2):
            wt = wtp.tile([128, KH, G, DS], bf16, tag="wt", name="wt")
            nc.sync.dma_start(wt[:], wsrc[:, h * KH:(h + 1) * KH, :, :])
            t01 = wrp.tile([128, KH, DS], bf16, tag="wred", name="wred")
            nc.vector.tensor_tensor(out=t01[:], in0=wt[:, :, 0, :],
                                    in1=wt[:, :, 1, :], op=ALU.add)
            t23 = wrp.tile([128, KH, DS], bf16, tag="wred", name="wred")
            nc.vector.tensor_tensor(out=t23[:], in0=wt[:, :, 2, :],
                                    in1=wt[:, :, 3, :], op=ALU.add)
            t45 = wrp.tile([128, KH, DS], bf16, tag="wred", name="wred")
            nc.vector.tensor_tensor(out=t45[:], in0=wt[:, :, 4, :],
                                    in1=wt[:, :, 5, :], op=ALU.add)
            nc.vector.tensor_tensor(out=t01[:], in0=t01[:], in1=t23[:],
                                    op=ALU.add)
            nc.vector.tensor_tensor(out=t45[:], in0=t45[:], in1=wt[:, :, 6, :],
                                    op=ALU.add)
            nc.vector.tensor_tensor(out=ws[:, h * KH:(h + 1) * KH, :],
                                    in0=t01[:], in1=t45[:], op=ALU.add)
        return xt, ws

    def group_matmuls(xt, ws, bias_cols):
        """raw = x @ Wsum (+ bias on the ACT copy); stage to SBUF bf16."""
        raws = []
        for ci, (m0, msz) in enumerate(CHUNKS):
            gp = gs0 if ci == 0 else gs1
            raw = gp.tile([msz, B], bf16, tag=f"gs{ci}", name=f"gs{ci}")
            for bs in range(NBS):
                sl = slice(bs * BSL, (bs + 1) * BSL)
                ps = ps_tile(msz)
                for kt in range(KT):
                    nc.tensor.matmul(
                        ps[:],
                        lhsT=ws[:, kt, m0:m0 + msz],
                        rhs=xt[:, kt, sl],
                        start=(kt == 0), stop=(kt == KT - 1))
                nc.scalar.activation(raw[:, sl], ps[:], ACTF.Identity,
                                     bias=bias_cols[ci][0:msz, :])
            raws.append(raw)
        return raws

    def fold_group(raws, i):
        """A += e_i * raw (e broadcast across partitions via sel matmul)."""
        for bs in range(NBS):
            sl = slice(bs * BSL, (bs + 1) * BSL)
            bc = ps_tile(128)
            nc.tensor.matmul(bc[:], lhsT=sel[:, i, :],
                             rhs=ex10[:, sl], start=True, stop=True)
            for ci, (m0, msz) in enumerate(CHUNKS):
                tmp = ctp.tile([128, BSL], f32, tag="ctmp", name="ctmp")
                nc.vector.tensor_tensor(out=tmp[0:msz, :], in0=raws[ci][:, sl],
                                        in1=bc[0:msz, :], op=ALU.mult)
                nc.vector.tensor_tensor(out=A[ci][:, sl], in0=A[ci][:, sl],
                                        in1=tmp[0:msz, :], op=ALU.add)

    # ================= uni branch first -> z, ssz partial, AR1 =========
    xib_t, wib_s = stream_group(xibT_d, wib_d)
    bib_cols = load_bias_cols(bib_d)
    bib_sc = []
    for ci, (m0, msz) in enumerate(CHUNKS):
        c2 = bsp.tile([128, 1], f32, tag="bsum", name="bsum")
        nc.vector.tensor_scalar_mul(c2[0:msz, :], bib_cols[ci][0:msz, :], 0.1 / 7.0)
        bib_sc.append(c2)
    for ci, (m0, msz) in enumerate(CHUNKS):
        for bs in range(NBS):
            sl = slice(bs * BSL, (bs + 1) * BSL)
            ps = ps_tile(msz)
            for kt in range(KT):
                nc.tensor.matmul(ps[:], lhsT=wib_s[:, kt, m0:m0 + msz],
                                 rhs=xib_t[:, kt, sl],
                                 start=(kt == 0), stop=(kt == KT - 1))
            # z = (raw + bias)*0.1/7 + 0.9*xu
            nc.scalar.activation(z[ci][:, sl], ps[:], ACTF.Identity,
                                 scale=0.1 / 7.0, bias=bib_sc[ci][0:msz, :])
            t9 = ctp.tile([128, BSL], f32, tag="ctmp", name="ctmp")
            nc.vector.tensor_scalar_mul(t9[0:msz, :], xu[ci][:, sl], 0.9)
            nc.vector.tensor_tensor(out=z[ci][:, sl], in0=z[ci][:, sl],
                                    in1=t9[0:msz, :], op=ALU.add)
    # ssz partial: sum_d z^2 -> arv slot 2 -> cc1_in
    for bs in range(NBS):
        sl = slice(bs * BSL, (bs + 1) * BSL)
        ps = ps_tile(1)
        for ci, (m0, msz) in enumerate(CHUNKS):
            sq = b1k.tile([128, B], f32r, tag="big1k", name="big1k")
            nc.scalar.square(sq[0:msz, 0:BSL], z[ci][:, sl])
            nc.tensor.matmul(ps[:], lhsT=onesb[0:msz, :],
                             rhs=sq[0:msz, 0:BSL],
                             start=(ci == 0), stop=(ci == len(CHUNKS) - 1))
        nc.scalar.copy(arv[:, B + bs * BSL:B + (bs + 1) * BSL], ps[:])
    nc.sync.dma_start(cc1_in[:], arv[:, B:2 * B])

    # ================= router groups (also feed h1) =================
    router_raws = {}
    for ri, g in enumerate(ROUTER_GS):
        xt, ws = stream_group(xT_d[g], w_d[g])
        bcols = load_bias_cols(bp_d[g])
        router_raws[g] = group_matmuls(xt, ws, bcols)
        if ri == 0:
            nc.gpsimd.collective_compute(
                "AllReduce", ALU.add,
                ins=[cc1_in.opt()], outs=[cc1_out.opt()],
                replica_groups=[list(range(NCORES))])
        # router h1 partial: h1[m] (+)= rw1[g-block].T @ xT[g]
        rt = rwp.tile([128, KT, BSL], bf16, tag="rwt", name="rwt")
        nc.sync.dma_start(rt[:], rw1_d[ri])
        for m in range(4):
            for bs in range(NBS):
                sl = slice(bs * BSL, (bs + 1) * BSL)
                ps = ps_tile(128)
                for kt in range(KT):
                    nc.tensor.matmul(
                        ps[:],
                        lhsT=rt[:, kt, m * 128:(m + 1) * 128],
                        rhs=xt[:, kt, sl],
                        start=(kt == 0), stop=(kt == KT - 1))
                if ri == 0:
                    nc.scalar.copy(h1[m][:, sl], ps[:])
                else:
                    nc.vector.tensor_tensor(out=h1[m][:, sl], in0=ps[:],
                                            in1=h1[m][:, sl], op=ALU.add)

    # ======= stream group 1 before the finalize (overlap its DMA/PE) ===
    g1 = STREAM_GS[3]
    xt1, ws1 = stream_group(xT_d[g1], w_d[g1])
    bcols1 = load_bias_cols(bp_d[g1])
    raws1 = group_matmuls(xt1, ws1, bcols1)

    # ================= router finalize =================
    def bn_act(tiles, out_tiles, nparts, func):
        """BatchNorm (training stats over free axis) + activation."""
        for t, to in zip(tiles, out_tiles):
            dump = b1k.tile([128, B], f32, tag="big1k", name="big1k")
            mnr = stp.tile([128, 1], f32, tag="stat", name="stat")
            nc.scalar.activation(dump[0:nparts, :], t[:], ACTF.Copy,
                                 accum_out=mnr[0:nparts, :])
            mn = stp.tile([128, 1], f32, tag="stat", name="stat")
            nc.scalar.mul(mn[0:nparts, :], mnr[0:nparts, :], 1.0 / B)
            sq = b1k.tile([128, B], f32, tag="big1k", name="big1k")
            ex2r = stp.tile([128, 1], f32, tag="stat", name="stat")
            nc.scalar.activation(sq[0:nparts, :], t[:], ACTF.Square,
                                 accum_out=ex2r[0:nparts, :])
            ex2 = stp.tile([128, 1], f32, tag="stat", name="stat")
            nc.scalar.mul(ex2[0:nparts, :], ex2r[0:nparts, :], 1.0 / B)
            var = stp.tile([128, 1], f32, tag="stat", name="stat")
            nc.vector.tensor_tensor(out=var[0:nparts, :], in0=mn[0:nparts, :],
                                    in1=mn[0:nparts, :], op=ALU.mult)
            nc.vector.tensor_tensor(out=var[0:nparts, :], in0=ex2[0:nparts, :],
                                    in1=var[0:nparts, :], op=ALU.subtract)
            nc.vector.tensor_scalar_add(var[0:nparts, :], var[0:nparts, :], EPS_BN)
            sd = stp.tile([128, 1], f32, tag="stat", name="stat")
            nc.scalar.sqrt(sd[0:nparts, :], var[0:nparts, :])
            rs = stp.tile([128, 1], f32, tag="stat", name="stat")
            nc.vector.reciprocal(rs[0:nparts, :], sd[0:nparts, :])
            nb = stp.tile([128, 1], f32, tag="stat", name="stat")
            nc.vector.tensor_tensor(out=nb[0:nparts, :], in0=mn[0:nparts, :],
                                    in1=rs[0:nparts, :], op=ALU.mult)
            nc.vector.tensor_scalar_mul(nb[0:nparts, :], nb[0:nparts, :], -1.0)
            nc.scalar.activation(to[:], t[:], func,
                                 bias=nb[0:nparts, :], scale=rs[0:nparts, :])

    bn_act(h1, h1b, 128, ACTF.Relu)
    for bs in range(NBS):
        sl = slice(bs * BSL, (bs + 1) * BSL)
        ps = ps_tile(100)
        for kt in range(4):
            nc.tensor.matmul(ps[:], lhsT=rw2[:, kt, :],
                             rhs=h1b[kt][:, sl],
                             start=(kt == 0), stop=(kt == 3))
        nc.scalar.copy(h2sb[:, sl], ps[:])
    bn_act([h2sb], [h2b], 100, ACTF.Tanh)
    for bs in range(NBS):
        sl = slice(bs * BSL, (bs + 1) * BSL)
        ps = ps_tile(11)
        nc.tensor.matmul(ps[:], lhsT=rw3[:],
                         rhs=h2b[:, sl], start=True, stop=True)
        sg = ctp.tile([128, BSL], f32, tag="ctmp", name="ctmp")
        nc.scalar.activation(sg[0:11, :], ps[:], ACTF.Sigmoid, bias=rb3[:], scale=1.0)
        nc.scalar.activation(ex10[:, sl], sg[0:11, :], ACTF.Exp, scale=10.0)

    # e10 row (no AR1 dependency): arv slot 3 <- 7*e10, then (7*e10)^2/8
    for bs in range(NBS):
        sl = slice(bs * BSL, (bs + 1) * BSL)
        bc = ps_tile(128)
        nc.tensor.matmul(bc[:], lhsT=sel[:, 10, :],
                         rhs=ex10[:, sl], start=True, stop=True)
        nc.scalar.copy(arv[:, 2 * B + bs * BSL:2 * B + (bs + 1) * BSL], bc[0:1, :])
    e10v = arv[:, 2 * B:3 * B]
    nc.vector.tensor_scalar_mul(e10v, e10v, 7.0)

    # ================= folds: router groups + g1, then stream rest =====
    for g in STREAM_GS[:3]:
        fold_group(router_raws[g], g)
    fold_group(raws1, g1)

    for gi, g in enumerate(STREAM_GS[4:]):
        xt, ws = stream_group(xT_d[g], w_d[g])
        bcols = load_bias_cols(bp_d[g])
        raws = group_matmuls(xt, ws, bcols)
        fold_group(raws, g)

    # AR1 result fetched on the gpsimd queue AFTER the last bias load --
    # nothing left behind it on gpsimd except the AR2 trigger, so the
    # wait for AR1 completion cannot stall the stream pipeline.
    nc.gpsimd.dma_start(arv[:, B:2 * B], cc1_out[:])
    s_v = arv[:, B:2 * B]
    nc.scalar.sqrt(s_v, s_v)
    nc.vector.tensor_scalar_max(s_v, s_v, EPS_NORM)
    nc.vector.reciprocal(uv[:], s_v)
    nc.vector.tensor_tensor(out=tv[:], in0=e10v, in1=uv[:], op=ALU.mult)
    # e10sq/8: each core contributes 1/8 so the AR2 sum restores it
    nc.vector.tensor_tensor(out=e10v, in0=e10v, in1=e10v, op=ALU.mult)
    nc.vector.tensor_scalar_mul(e10v, e10v, 0.125)

    # ========= tail: q_loc = |A|^2 + t*(2 A.z) + (7e10)^2/8 -> AR2 ======
    for bs in range(NBS):
        sl = slice(bs * BSL, (bs + 1) * BSL)
        psa = ps_tile(1)
        psc = ps_tile(1)
        for ci, (m0, msz) in enumerate(CHUNKS):
            sqa = b1k.tile([128, B], f32r, tag="big1k", name="big1k")
            nc.scalar.square(sqa[0:msz, 0:BSL], A[ci][:, sl])
            nc.tensor.matmul(psa[:], lhsT=onesb[0:msz, :],
                             rhs=sqa[0:msz, 0:BSL],
                             start=(ci == 0), stop=(ci == len(CHUNKS) - 1))
            cza = b1k.tile([128, B], f32r, tag="big1k", name="big1k")
            nc.vector.tensor_tensor(out=cza[0:msz, 0:BSL], in0=A[ci][:, sl],
                                    in1=z[ci][:, sl], op=ALU.mult)
            nc.tensor.matmul(psc[:], lhsT=twosb[0:msz, :],
                             rhs=cza[0:msz, 0:BSL],
                             start=(ci == 0), stop=(ci == len(CHUNKS) - 1))
        qt = ctp.tile([128, BSL], f32, tag="ctmp", name="ctmp")
        nc.vector.tensor_tensor(out=qt[0:1, :], in0=psc[:], in1=tv[:, sl],
                                op=ALU.mult)
        nc.vector.tensor_tensor(out=qt[0:1, :], in0=qt[0:1, :], in1=psa[:],
                                op=ALU.add)
        nc.vector.tensor_tensor(out=arv[:, bs * BSL:(bs + 1) * BSL],
                                in0=qt[0:1, :],
                                in1=arv[:, 2 * B + bs * BSL:2 * B + (bs + 1) * BSL],
                                op=ALU.add)
    nc.sync.dma_start(cc2_in[:], arv[:, 0:B])
    nc.gpsimd.collective_compute(
        "AllReduce", ALU.add,
        ins=[cc2_in.opt()], outs=[cc2_out.opt()],
        replica_groups=[list(range(NCORES))])

    # P = A + t*z, computed while AR2 is in flight
    for bs in range(NBS):
        sl = slice(bs * BSL, (bs + 1) * BSL)
        btv = ps_tile(128)
        nc.tensor.matmul(btv[:], lhsT=ones1[:],
                         rhs=tv[:, sl], start=True, stop=True)
        for ci, (m0, msz) in enumerate(CHUNKS):
            t2 = ctp.tile([128, BSL], f32, tag="ctmp", name="ctmp")
            nc.vector.tensor_tensor(out=t2[0:msz, :], in0=z[ci][:, sl],
                                    in1=btv[0:msz, :], op=ALU.mult)
            nc.vector.tensor_tensor(out=A[ci][:, sl], in0=A[ci][:, sl],
                                    in1=t2[0:msz, :], op=ALU.add)

    nc.sync.dma_start(arv[:, 0:B], cc2_out[:])

    # q -> u = 1/max(sqrt(q), eps); out = P*u
    a_v = arv[:, 0:B]
    nc.scalar.sqrt(a_v, a_v)
    nc.vector.tensor_scalar_max(a_v, a_v, EPS_NORM)
    nc.vector.reciprocal(uv[:], a_v)
    for bs in range(NBS):
        sl = slice(bs * BSL, (bs + 1) * BSL)
        bu = ps_tile(128)
        nc.tensor.matmul(bu[:], lhsT=ones1[:],
                         rhs=uv[:, sl], start=True, stop=True)
        for ci, (m0, msz) in enumerate(CHUNKS):
            nc.vector.tensor_tensor(out=outsb[ci][:, sl], in0=A[ci][:, sl],
                                    in1=bu[0:msz, :], op=ALU.mult)
    for ci, (m0, msz) in enumerate(CHUNKS):
        nc.sync.dma_start(outT_d[m0:m0 + msz, :], outsb[ci][:])

    for p in reversed(list(pools.values())):
        p.__exit__(None, None, None)


def _build_nc():
    nc = bacc.Bacc("TRN2", target_bir_lowering=False, debug=False,
                   num_devices=NCORES)
    with tile.TileContext(nc) as tc:
        with nc.allow_low_precision(reason="bf16 streams / f32r reductions are intentional"):
            _emit(nc, tc)
    nc.compile()
    return nc


def _as_bf16(a):
    return np.ascontiguousarray(a.astype(ml_dtypes.bfloat16))


def _host_prep(inputs):
    x_enc = np.asarray(inputs["x_enc"], dtype=np.float32)
    x_ib = np.asarray(inputs["x_ib"], dtype=np.float32)
    x_uni = np.asarray(inputs["x_uni"], dtype=np.float32)
    W_proj = np.asarray(inputs["W_proj"], dtype=np.float32)
    b_proj = np.asarray(inputs["b_proj"], dtype=np.float32)
    W_ib = np.asarray(inputs["W_ib"], dtype=np.float32)
    b_ib = np.asarray(inputs["b_ib"], dtype=np.float32)

    # x_enc [N,B,K] -> [N, 128, KT, B] partition-major bf16
    xT = _as_bf16(x_enc.transpose(0, 2, 1).reshape(N, KT, 128, B).transpose(0, 2, 1, 3))
    # x_ib [B,K] -> [128, KT, B]
    xibT = _as_bf16(x_ib.T.reshape(KT, 128, B).transpose(1, 0, 2))
    sel = np.zeros((11, 11, 128), dtype=np.float32)
    for q in range(11):
        sel[q, q, :] = 1.0
    rb3 = np.ascontiguousarray(np.asarray(inputs["r_b3"], np.float32).reshape(11, 1))
    # r_w1 [3072, 512] -> [3, 128, KT, 512]
    rw1 = _as_bf16(np.asarray(inputs["r_w1"], np.float32)
                   .reshape(3, KT, 128, BSL).transpose(0, 2, 1, 3))
    rw2 = _as_bf16(np.asarray(inputs["r_w2"], np.float32)
                   .reshape(4, 128, 100).transpose(1, 0, 2))
    rw3 = _as_bf16(np.asarray(inputs["r_w3"], np.float32))
    ones_host = np.ones((128, 130), dtype=np.float32)
    ones_host[:, 1] = 2.0

    in_maps = []
    for c in range(NCORES):
        ds = slice(c * DS, (c + 1) * DS)
        # W_proj [N,G,K,D] ds-slice -> [N, 128, KT, G, DS]
        wc = _as_bf16(W_proj[:, :, :, ds].reshape(N, G, KT, 128, DS)
                      .transpose(0, 3, 2, 1, 4))
        wibc = _as_bf16(W_ib[:, :, ds].reshape(G, KT, 128, DS)
                        .transpose(2, 1, 0, 3))
        in_maps.append({
            "xT": xT,
            "xibT": xibT,
            "w": wc,
            "wib": wibc,
            "xuT": _as_bf16(x_uni[:, ds].T),
            "bp": _as_bf16(b_proj[:, :, ds].transpose(0, 2, 1)),
            "bib": _as_bf16(b_ib[:, ds].T),
            "rw1": rw1,
            "rw2": rw2,
            "rw3": rw3,
            "rb3": rb3,
            "sel": sel,
            "onesd": ones_host,
        })
    return in_maps


def kernel(**inputs):
    global LAST_RESULTS
    if "nc" not in _NC_CACHE:
        _NC_CACHE["nc"] = _build_nc()
    nc = _NC_CACHE["nc"]
    in_maps = _host_prep(inputs)
    res = run_bass_kernel_spmd(nc, in_maps, list(range(NCORES)))
    LAST_RESULTS = res
    full = np.concatenate([res.results[c]["outT"] for c in range(NCORES)], axis=0)
    return np.ascontiguousarray(full.T)
